# revision 39
# baseline (speedup 1.0000x reference)
"""BPNN (Behler-Parrinello NN) Trainium2 kernel, 8-way SPMD.

Sharding: atoms sharded at image boundaries across 8 NeuronCores; per-element
MLPs routed by sorting each shard's atoms by element (host index prep).
fprime COO entries are sharded by source atom's core; products are formed
on-device by routing g-elements to padded, column-sorted slot positions via
GPSIMD local_scatter + PE transposes, then reduced and ReduceScatter-summed
across cores (each core outputs its 1/8 force slice; host concatenates).

kernel(**inputs) takes FULL unsharded inputs, returns (energy, forces).
"""
import sys

sys.path.insert(0, "/opt/trn_rl_repo")

import numpy as np
import ml_dtypes

from concourse import bass, bacc, mybir
import concourse.tile as tile
from concourse.tile import add_dep_helper
from concourse.bass_utils import run_bass_kernel_spmd
from concourse.bass import IndirectOffsetOnAxis

BF16 = ml_dtypes.bfloat16
F32 = np.float32
NC = 8
PAD = 6       # slots per force cell
ELEMENTS = np.array([1, 6, 8, 29])


def _even(x):
    return int(x) + (int(x) & 1)


def _host_prepare(inputs):
    fp = np.asarray(inputs["fingerprints"], dtype=np.float32)
    an = np.asarray(inputs["atomic_numbers"])
    img = np.asarray(inputs["image_idx"])
    rows = np.asarray(inputs["fprime_rows"])
    cols = np.asarray(inputs["fprime_cols"])
    vals = np.asarray(inputs["fprime_vals"], dtype=np.float32)

    N, D = fp.shape
    E = 4
    NI = int(img.max()) + 1
    CTOT = 3 * N
    elem_idx = np.searchsorted(ELEMENTS, an)

    # ---- shard atoms at image boundaries ----
    img_starts = np.searchsorted(img, np.arange(NI))
    bounds = [0]
    for t in ((np.arange(1, NC) * N) // NC):
        i = np.searchsorted(img_starts, t)
        cand = []
        if i > 0:
            cand.append(int(img_starts[i - 1]))
        if i < NI:
            cand.append(int(img_starts[i]))
        bounds.append(min(cand, key=lambda x: abs(x - t)))
    bounds.append(N)
    bounds = np.array(bounds)
    assert np.all(np.diff(bounds) > 0), "image-aligned shard bounds degenerate"

    cores = []
    for k in range(NC):
        lo, hi = int(bounds[k]), int(bounds[k + 1])
        atoms = np.arange(lo, hi)
        e = elem_idx[lo:hi]
        order = np.argsort(e, kind="stable")
        cores.append(dict(atoms=atoms[order], elems=e[order], lo=lo, hi=hi))

    T_e = []
    for ei in range(E):
        mx = max(int((c["elems"] == ei).sum()) for c in cores)
        T_e.append(_even((mx + 127) // 128))
    T_e = np.array(T_e)
    T = int(T_e.sum())
    PADN = T * 128

    tile_elem = np.concatenate([np.full(T_e[ei], ei) for ei in range(E)])

    atom_core = np.empty(N, dtype=np.int64)
    atom_pos = np.empty(N, dtype=np.int64)
    IMGMAX = 0
    for k, c in enumerate(cores):
        padded = np.full(PADN, -1, dtype=np.int64)
        pos = 0
        for ei in range(E):
            seg = c["atoms"][c["elems"] == ei]
            padded[pos : pos + len(seg)] = seg
            pos += T_e[ei] * 128
        c["padded_atoms"] = padded
        li = np.full(PADN, -1, dtype=np.int64)
        m = padded >= 0
        li[m] = img[padded[m]] - img[c["lo"]]
        c["img_local"] = li
        c["img_base"] = int(img[c["lo"]])
        c["n_img"] = int(img[c["hi"] - 1] - img[c["lo"]] + 1)
        IMGMAX = max(IMGMAX, c["n_img"])
        atom_core[padded[m]] = k
        atom_pos[padded[m]] = np.nonzero(m)[0]

    IMG = max(64, _even(IMGMAX))
    assert IMG <= 128, f"too many images per shard: {IMGMAX}"

    QC = (CTOT + 127) // 128
    QC = ((QC + 3) // 4) * 4
    CHUNK_Q = QC // 4
    SLOTC = QC * PAD

    # ---- force entries: source-sharded routing construction ----
    a_src = rows // D
    d_src = rows % D
    core_of = atom_core[a_src]
    pos_of = atom_pos[a_src]

    TSC = 14  # staging tiles per chunk
    percore = []
    for k in range(NC):
        sel = np.nonzero(core_of == k)[0]
        c_k = cols[sel]
        # order by dest cell for crank
        o1 = np.lexsort((sel, c_k))
        sel = sel[o1]
        c_k = c_k[o1]
        d_k = d_src[sel]
        ap_k = pos_of[sel]
        n = len(sel)
        newc = np.r_[True, np.diff(c_k) != 0]
        firstc = np.nonzero(newc)[0]
        crank = np.arange(n) - firstc[np.cumsum(newc) - 1]
        # duplicate rank within source element (d, apos)
        o2 = np.lexsort((np.arange(n), ap_k, d_k))
        sk = d_k[o2].astype(np.int64) * PADN + ap_k[o2]
        news = np.r_[True, np.diff(sk) != 0]
        firsts = np.nonzero(news)[0]
        srank = np.empty(n, dtype=np.int64)
        srank[o2] = np.arange(n) - firsts[np.cumsum(news) - 1]
        # staging tile rank within (chunk, d, p_dst)
        p_dst = c_k % 128
        q_dst = c_k // 128
        chunk = q_dst // CHUNK_Q
        o3 = np.lexsort((np.arange(n), p_dst, d_k, chunk))
        tk = ((chunk[o3] * 128 + d_k[o3]) * 128 + p_dst[o3]).astype(np.int64)
        newt = np.r_[True, np.diff(tk) != 0]
        firstt = np.nonzero(newt)[0]
        trank = np.empty(n, dtype=np.int64)
        trank[o3] = np.arange(n) - firstt[np.cumsum(newt) - 1]

        main = (crank < PAD) & (srank < 2) & (trank < TSC)
        percore.append(dict(
            sel=sel, c=c_k, d=d_k, ap=ap_k, crank=crank, srank=srank,
            trank=trank, p=p_dst, q=q_dst, chunk=chunk, main=main,
        ))

    # compact region sizes per (half, sr) (global max over cores)
    NGH = max(1, (PADN // 256) // 2)
    SPLIT_AP = NGH * 256
    LAh = [2, 2]
    LBh = [2, 2]
    for pc in percore:
        m = pc["main"]
        half = (pc["ap"] >= SPLIT_AP).astype(np.int64)
        for h in (0, 1):
            for sr in (0, 1):
                mm = m & (pc["srank"] == sr) & (half == h)
                if mm.any():
                    cnt = int(np.bincount(pc["d"][mm], minlength=128).max())
                    if sr == 0:
                        LAh[h] = max(LAh[h], cnt)
                    else:
                        LBh[h] = max(LBh[h], cnt)
    LAh = [min(2046, _even(x + 2)) for x in LAh]
    LBh = [min(2046, _even(x + 2)) for x in LBh]

    # ---- per-core streams ----
    in_maps = []
    core_meta = []
    w = {k: np.asarray(inputs[k], dtype=np.float32) for k in
         ("W0", "b0", "W1", "b1", "W2", "b2")}
    H = w["W0"].shape[2]
    HC = H // 128
    assert H % 128 == 0 and D == 128

    w0sb = np.zeros((128, E * HC * 128), dtype=BF16)
    w1sb = np.zeros((128, E * HC * HC * 128), dtype=BF16)
    w1tsb = np.zeros((128, E * HC * HC * 128), dtype=BF16)
    w0tsb = np.zeros((128, E * HC * 128), dtype=BF16)
    w2col = np.zeros((128, E * HC), dtype=BF16)
    w2scal = np.zeros((128, E * HC), dtype=F32)
    b0sb = np.zeros((128, E * HC), dtype=F32)
    b1sb = np.zeros((128, E * HC), dtype=F32)
    b2bc = np.zeros((128, E), dtype=F32)
    for e in range(E):
        for m in range(HC):
            w0sb[:, (e * HC + m) * 128 : (e * HC + m + 1) * 128] = \
                w["W0"][e][:, m * 128 : (m + 1) * 128].astype(BF16)
            w0tsb[:, (e * HC + m) * 128 : (e * HC + m + 1) * 128] = \
                w["W0"][e][:, m * 128 : (m + 1) * 128].T.astype(BF16)
            w2col[:, e * HC + m] = w["W2"][e][m * 128 : (m + 1) * 128, 0].astype(BF16)
            w2scal[:, e * HC + m] = w["W2"][e][m * 128 : (m + 1) * 128, 0]
            b0sb[:, e * HC + m] = w["b0"][e][m * 128 : (m + 1) * 128]
            b1sb[:, e * HC + m] = w["b1"][e][m * 128 : (m + 1) * 128]
            for kc in range(HC):
                col = ((e * HC + kc) * HC + m) * 128
                w1sb[:, col : col + 128] = \
                    w["W1"][e][kc * 128 : (kc + 1) * 128,
                               m * 128 : (m + 1) * 128].astype(BF16)
                w1tsb[:, col : col + 128] = \
                    w["W1"][e][m * 128 : (m + 1) * 128,
                               kc * 128 : (kc + 1) * 128].T.astype(BF16)
        b2bc[:, e] = w["b2"][e][0]
    ident = np.eye(128, dtype=BF16)
    iota_img = np.tile(np.arange(IMG, dtype=F32)[None, :], (128, 1))

    # ---- per-core main + mini routing assignment ----
    prep = []
    LAtot = [LAh[0], LAh[1]]
    TS_MINI, PAD2, P_MINI = 1, 1, 1
    for k in range(NC):
        c = cores[k]
        pc = percore[k]
        mm = pc["main"]
        d = pc["d"][mm]; ap = pc["ap"][mm]; sr = pc["srank"][mm]
        hh = (ap >= SPLIT_AP).astype(np.int64)
        regid = hh * 2 + sr
        # compact position: rank within (region, partition d), ordered by ap
        o4 = np.lexsort((ap, d, regid))
        key = regid[o4] * 128 + d[o4]
        newk = np.r_[True, np.diff(key) != 0]
        firstk = np.nonzero(newk)[0]
        cpos_rel = np.empty(len(key), dtype=np.int64)
        cpos_rel[o4] = np.arange(len(key)) - firstk[np.cumsum(newk) - 1]
        capv = np.array([LAh[0], LBh[0], LAh[1], LBh[1]])
        fit = cpos_rel < capv[regid]

        # mini set: non-main entries + main entries that lost capacity
        mini_sel = np.r_[np.nonzero(~mm)[0], np.nonzero(mm)[0][~fit]]
        md = pc["d"][mini_sel]; map_ = pc["ap"][mini_sel]
        mc = pc["c"][mini_sel]; mp = pc["p"][mini_sel]; mq = pc["q"][mini_sel]
        nm = len(mini_sel)

        # source slot lookup: fit entries define (d,ap)->cpos per region
        mA_ = fit & (sr == 0)
        akey = d[mA_].astype(np.int64) * PADN + ap[mA_]
        apos_ = cpos_rel[mA_]
        sa = np.argsort(akey)
        akey_s, apos_s = akey[sa], apos_[sa]
        mB_ = fit & (sr == 1)
        bkey = d[mB_].astype(np.int64) * PADN + ap[mB_]
        bpos_ = cpos_rel[mB_]
        sb = np.argsort(bkey)
        bkey_s, bpos_s = bkey[sb], bpos_[sb]

        mkey = md.astype(np.int64) * PADN + map_
        mhalf = (map_ >= SPLIT_AP).astype(np.int64)
        ia = np.searchsorted(akey_s, mkey)
        hasA = (ia < len(akey_s)) & (akey_s[np.minimum(ia, len(akey_s) - 1)] == mkey)
        ib = np.searchsorted(bkey_s, mkey)
        hasB = (ib < len(bkey_s)) & (bkey_s[np.minimum(ib, len(bkey_s) - 1)] == mkey)

        mcpos = np.zeros(nm, dtype=np.int64)
        mreg = np.zeros(nm, dtype=np.int64)
        mcpos[hasA] = apos_s[ia[hasA]]
        mreg[hasA] = mhalf[hasA] * 2
        mcpos[hasB & ~hasA] = bpos_s[ib[hasB & ~hasA]]
        mreg[hasB & ~hasA] = mhalf[hasB & ~hasA] * 2 + 1
        # sources with BOTH A and B slots: alternate copies across the two
        # slots (halves the per-slot duplicate rank -> fewer ls1m passes)
        both = hasA & hasB
        if both.any():
            o_ab = np.lexsort((np.arange(nm), mkey))
            kab = mkey[o_ab]
            newab = np.r_[True, np.diff(kab) != 0]
            fab = np.nonzero(newab)[0]
            cprank = np.arange(nm) - fab[np.cumsum(newab) - 1]
            crk = np.empty(nm, dtype=np.int64)
            crk[o_ab] = cprank
            useB = both & (crk % 2 == 1)
            mcpos[useB] = bpos_s[ib[useB]]
            mreg[useB] = mhalf[useB] * 2 + 1
        # new ext-A slots for mini-only sources (one per unique source)
        need = ~(hasA | hasB)
        if need.any():
            nk = mkey[need]
            nh = mhalf[need]
            o7 = np.lexsort((nk,))
            nk_s = nk[o7]
            newu = np.r_[True, np.diff(nk_s) != 0]
            uid = np.cumsum(newu) - 1
            ukeys = nk_s[newu]
            ud = (ukeys // PADN).astype(np.int64)
            uap = (ukeys % PADN).astype(np.int64)
            uh = (uap >= SPLIT_AP).astype(np.int64)
            cntA0 = np.bincount(d[mA_ & (hh[mA_] == 0) if False else mA_][
                (ap[mA_] < SPLIT_AP)], minlength=128)
            cntA1 = np.bincount(d[mA_][(ap[mA_] >= SPLIT_AP)], minlength=128)
            # rank among unique-ext sources of the same (half, d)
            keyhd = uh * 128 + ud
            o8 = np.argsort(keyhd, kind="stable")
            k_s = keyhd[o8]
            newd = np.r_[True, np.diff(k_s) != 0]
            fd = np.nonzero(newd)[0]
            extrank = np.arange(len(k_s)) - fd[np.cumsum(newd) - 1]
            upos = np.empty(len(ud), dtype=np.int64)
            upos[o8] = extrank
            upos = np.where(uh == 0, cntA0[ud], cntA1[ud]) + upos
            LAtot[0] = max(LAtot[0], int((upos[uh == 0] + 1).max())
                           if (uh == 0).any() else 0)
            LAtot[1] = max(LAtot[1], int((upos[uh == 1] + 1).max())
                           if (uh == 1).any() else 0)
            tmpn = np.empty(need.sum(), dtype=np.int64)
            tmpn[o7] = upos[uid]
            mcpos[need] = tmpn
            tmpr = np.empty(need.sum(), dtype=np.int64)
            tmpr[o7] = uh[uid] * 2
            mreg[need] = tmpr
            ext_d = ud
            ext_ap = uap
            ext_pos = upos
        else:
            ext_d = np.empty(0, dtype=np.int64)
            ext_ap = np.empty(0, dtype=np.int64)
            ext_pos = np.empty(0, dtype=np.int64)

        # mini ranks
        o9 = np.lexsort((np.arange(nm), mc))
        newc2 = np.r_[True, np.diff(mc[o9]) != 0]
        fco = np.nonzero(newc2)[0]
        mcrank = np.empty(nm, dtype=np.int64)
        mcrank[o9] = np.arange(nm) - fco[np.cumsum(newc2) - 1]
        o11 = np.lexsort((np.arange(nm), mcpos, mreg, md))
        k11 = (md[o11] * 4 + mreg[o11]) * 4096 + mcpos[o11]
        news2 = np.r_[True, np.diff(k11) != 0]
        fs2 = np.nonzero(news2)[0]
        msrank = np.empty(nm, dtype=np.int64)
        msrank[o11] = np.arange(nm) - fs2[np.cumsum(news2) - 1]
        o10 = np.lexsort((np.arange(nm), mp, md, msrank))
        k10 = (msrank[o10] * 128 + md[o10]) * 128 + mp[o10]
        newt2 = np.r_[True, np.diff(k10) != 0]
        ft2 = np.nonzero(newt2)[0]
        mtrank = np.empty(nm, dtype=np.int64)
        mtrank[o10] = np.arange(nm) - ft2[np.cumsum(newt2) - 1]

        if nm:
            TS_MINI = max(TS_MINI, int(mtrank.max()) + 1)
            PAD2 = max(PAD2, int(mcrank.max()) + 1)
            P_MINI = max(P_MINI, int(msrank.max()) + 1)
        # per-pass staging tile requirement
        tsj = np.ones(64, dtype=np.int64)
        for j in range(int(msrank.max()) + 1 if nm else 0):
            mj = msrank == j
            if mj.any():
                tsj[j] = max(tsj[j], int(mtrank[mj].max()) + 1)
        prep_tsj = tsj

        prep.append(dict(
            fit=fit, cpos_rel=cpos_rel, ext_d=ext_d, ext_ap=ext_ap,
            ext_pos=ext_pos, mini_sel=mini_sel, md=md, map_=map_, mc=mc,
            mp=mp, mq=mq, mcpos=mcpos, mreg=mreg, mcrank=mcrank,
            mtrank=mtrank, msrank=msrank, tsj=prep_tsj,
        ))

    LA2h = [min(2046, _even(x + 2)) for x in LAtot]
    assert max(LAtot) <= 2044, f"compact A region overflow {LAtot}"
    assert TS_MINI <= 15, TS_MINI
    # region order: [h0A, h0B, h1A, h1B]; regid = half*2 + sr
    REG_OFF = np.array([0, LA2h[0], LA2h[0] + LBh[0],
                        LA2h[0] + LBh[0] + LA2h[1]])
    LCTOT = LA2h[0] + LBh[0] + LA2h[1] + LBh[1]
    TS_LIST = [max(int(pr["tsj"][j]) for pr in prep) for j in range(P_MINI)]
    STG_OFF = np.r_[0, np.cumsum(TS_LIST)] * 128
    SMW_G = int(STG_OFF[-1])

    for k in range(NC):
        c = cores[k]
        pa = c["padded_atoms"]
        m = pa >= 0
        fpT = np.zeros((128, PADN), dtype=BF16)
        fpT[:, m] = fp[pa[m]].T.astype(BF16)
        imgsb = c["img_local"].reshape(T, 128).T.astype(F32)

        pc = percore[k]
        pr = prep[k]
        mm = pc["main"]
        fit = pr["fit"]
        cpos_rel = pr["cpos_rel"]
        d = pc["d"][mm]; ap = pc["ap"][mm]; sr = pc["srank"][mm]
        ch = pc["chunk"][mm]; p = pc["p"][mm]; q = pc["q"][mm]
        cr = pc["crank"][mm]; tr = pc["trank"][mm]
        v_main = vals[pc["sel"][mm]]
        hh2 = (ap >= SPLIT_AP).astype(np.int64)
        cpos = REG_OFF[hh2 * 2 + sr] + cpos_rel

        ls0a_idx = np.full((128, PADN), -1, dtype=np.int16)
        ls0b_idx = np.full((128, PADN), -1, dtype=np.int16)
        mA = fit & (sr == 0)
        mB = fit & (sr == 1)
        ls0a_idx[d[mA], ap[mA]] = cpos_rel[mA].astype(np.int16)
        ls0a_idx[pr["ext_d"], pr["ext_ap"]] = pr["ext_pos"].astype(np.int16)
        ls0b_idx[d[mB], ap[mB]] = cpos_rel[mB].astype(np.int16)

        ls1_idx = np.full((128, 4 * LCTOT), -1, dtype=np.int16)
        ls2_idx = np.full((128, 4 * TSC * 128), -1, dtype=np.int16)
        vslots = np.zeros((128, SLOTC), dtype=BF16)
        f = fit
        ls1_idx[d[f], ch[f] * LCTOT + cpos[f]] = (tr[f] * 128 + p[f]).astype(np.int16)
        ls2_idx[p[f], (ch[f] * TSC + tr[f]) * 128 + d[f]] = \
            (q[f] % CHUNK_Q * PAD + cr[f]).astype(np.int16)
        vslots[p[f], q[f] * PAD + cr[f]] = v_main[f].astype(BF16)

        # mini streams
        md, mp, mq = pr["md"], pr["mp"], pr["mq"]
        mcpos, mcrank, mtrank, msrank = (pr["mcpos"], pr["mcrank"],
                                         pr["mtrank"], pr["msrank"])
        mcp = mcpos + REG_OFF[pr["mreg"]]
        SMW = SMW_G
        ls1m_idx = np.full((128, P_MINI * LCTOT), -1, dtype=np.int16)
        ls2m_idx = np.full((128, PAD2 * SMW), -1, dtype=np.int16)
        v2 = np.zeros((128, PAD2 * QC), dtype=BF16)
        if len(md):
            msr = pr["msrank"]; mtr = pr["mtrank"]; mcr = pr["mcrank"]
            mv = vals[pc["sel"][pr["mini_sel"]]]
            ls1m_idx[md, msr * LCTOT + mcp] = \
                (mtr * 128 + mp).astype(np.int16)
            ls2m_idx[mp, mcr * SMW + STG_OFF[msr] + mtr * 128 + md] = \
                mq.astype(np.int16)
            v2[mp, mcr * QC + mq] = mv.astype(BF16)

        in_maps.append(dict(
            fpT=fpT, imgsb=imgsb, iota_img=iota_img, ident=ident,
            w0sb=w0sb, w1sb=w1sb, w1tsb=w1tsb, w0tsb=w0tsb,
            w2col=w2col, w2scal=w2scal, b0sb=b0sb, b1sb=b1sb, b2bc=b2bc,
            ls0a_idx=ls0a_idx, ls0b_idx=ls0b_idx,
            ls1_idx=ls1_idx, ls2_idx=ls2_idx, vslots=vslots,
            ls1m_idx=ls1m_idx, ls2m_idx=ls2m_idx, v2=v2,
        ))
        core_meta.append(dict(img_base=c["img_base"], n_img=c["n_img"]))

    cfg = dict(
        N=N, D=D, H=H, E=E, NI=NI, T=T, PADN=PADN, HC=HC,
        tile_elem=tuple(int(x) for x in tile_elem),
        IMG=IMG, QC=QC, CHUNK_Q=CHUNK_Q, SLOTC=SLOTC, TSC=TSC,
        LA2h=tuple(LA2h), LBh=tuple(LBh), LCTOT=LCTOT,
        SPLIT_AP=SPLIT_AP, REG_OFF=tuple(int(x) for x in REG_OFF),
        TS_MINI=TS_MINI, PAD2=PAD2, P_MINI=P_MINI,
        TS_LIST=tuple(TS_LIST), SMW=SMW_G,
        in_maps=in_maps, core_meta=core_meta,
    )
    return cfg


def _build(cfg, num_devices=NC, no_collective=False):
    PADN, T, HC, IMG = cfg["PADN"], cfg["T"], cfg["HC"], cfg["IMG"]
    QC, SLOTC, TSC, LCTOT = (cfg["QC"], cfg["SLOTC"], cfg["TSC"],
                             cfg["LCTOT"])
    LA2h, LBh, E = cfg["LA2h"], cfg["LBh"], cfg["E"]
    SPLIT_AP, REG_OFF = cfg["SPLIT_AP"], cfg["REG_OFF"]
    TS_MINI, PAD2, P_MINI = cfg["TS_MINI"], cfg["PAD2"], cfg["P_MINI"]
    bf = mybir.dt.bfloat16
    f32 = mybir.dt.float32
    i16 = mybir.dt.int16
    i32 = mybir.dt.int32
    AF = mybir.ActivationFunctionType
    OP = mybir.AluOpType

    nc = bacc.Bacc("TRN2", target_bir_lowering=False, debug=False,
                   num_devices=num_devices)

    def din(name, shape, dtype):
        return nc.dram_tensor(name, shape, dtype, kind="ExternalInput")

    fpT_d = din("fpT", [128, PADN], bf)
    imgsb_d = din("imgsb", [128, T], f32)
    iota_d = din("iota_img", [128, IMG], f32)
    ident_d = din("ident", [128, 128], bf)
    w0_d = din("w0sb", [128, E * HC * 128], bf)
    w1_d = din("w1sb", [128, E * HC * HC * 128], bf)
    w1t_d = din("w1tsb", [128, E * HC * HC * 128], bf)
    w0t_d = din("w0tsb", [128, E * HC * 128], bf)
    w2c_d = din("w2col", [128, E * HC], bf)
    w2s_d = din("w2scal", [128, E * HC], f32)
    b0_d = din("b0sb", [128, E * HC], f32)
    b1_d = din("b1sb", [128, E * HC], f32)
    b2_d = din("b2bc", [128, E], f32)
    ls0a_d = din("ls0a_idx", [128, PADN], i16)
    ls0b_d = din("ls0b_idx", [128, PADN], i16)
    ls1_d = din("ls1_idx", [128, 4 * LCTOT], i16)
    ls2_d = din("ls2_idx", [128, 4 * TSC * 128], i16)
    vsl_d = din("vslots", [128, SLOTC], bf)
    ls1m_d = din("ls1m_idx", [128, cfg["P_MINI"] * LCTOT], i16)
    ls2m_d = din("ls2m_idx", [128, cfg["PAD2"] * cfg["SMW"]], i16)
    v2_d = din("v2", [128, cfg["PAD2"] * QC], bf)

    part_dram = nc.dram_tensor("part_dram", [128 * QC, 1], f32)
    ar_out = nc.dram_tensor("ar_out", [(128 // num_devices) * QC, 1], f32)
    energy_out = nc.dram_tensor("energy_out", [1, IMG], f32, kind="ExternalOutput")
    forces_out = nc.dram_tensor("forces_out", [128 // num_devices, QC], f32,
                                kind="ExternalOutput")

    tile_elem = cfg["tile_elem"]
    NG = T // 2

    with tile.TileContext(nc) as tc:
        with (
            tc.tile_pool(name="persist", bufs=1) as pp,
            tc.tile_pool(name="grp", bufs=3) as gp,
            tc.tile_pool(name="idxp", bufs=2) as ixp,
            tc.tile_pool(name="ps2", bufs=2, space="PSUM") as ps2,
            tc.tile_pool(name="ps1", bufs=1, space="PSUM") as ps1,
        ):
            def pload(name, dram, shape, dtype, pool=None, tag=None):
                t = (pool or pp).tile(shape, dtype, tag=tag or name)
                nc.sync.dma_start(t[:], dram.ap())
                return t

            fpT = pload("fpT", fpT_d, [128, PADN], bf, tag="bigA")
            w0 = pload("w0", w0_d, [128, E * HC * 128], bf)
            w1 = pload("w1", w1_d, [128, E * HC * HC * 128], bf)
            w1t = pload("w1t", w1t_d, [128, E * HC * HC * 128], bf)
            w0t = pload("w0t", w0t_d, [128, E * HC * 128], bf)
            w2c = pload("w2c", w2c_d, [128, E * HC], bf)
            w2s = pload("w2s", w2s_d, [128, E * HC], f32)
            b0 = pload("b0", b0_d, [128, E * HC], f32)
            b1 = pload("b1", b1_d, [128, E * HC], f32)
            b2 = pload("b2", b2_d, [128, E], f32)
            imgs = pload("imgs", imgsb_d, [128, T], f32)
            iota = pload("iota", iota_d, [128, IMG], f32)
            ident = pload("ident", ident_d, [128, 128], bf)

            # W1wT = W1T * w2 (per hout-chunk partition scalar)
            w1wt = pp.tile([128, E * HC * HC * 128], bf, tag="w1wt")
            for e in range(E):
                for kc in range(HC):
                    col = (e * HC + kc) * HC * 128
                    nc.vector.tensor_scalar(
                        out=w1wt[:, col : col + HC * 128],
                        in0=w1t[:, col : col + HC * 128],
                        scalar1=w2s[:, e * HC + kc : e * HC + kc + 1],
                        scalar2=None, op0=OP.mult)

            g_h = [pp.tile([128, SPLIT_AP], bf, tag="gh0", name="gh0"),
                   pp.tile([128, PADN - SPLIT_AP], bf, tag="gh1", name="gh1")]
            o_col = pp.tile([128, T], bf, tag="o_col")
            ps_E = ps1.tile([1, IMG], f32, tag="ps_E")

            GA = 256
            for g_i in range(NG):
                e = tile_elem[2 * g_i]
                a0 = g_i * GA
                X = fpT[:, a0 : a0 + GA]
                h0 = gp.tile([128, HC, GA], bf, tag="h0")
                h1 = gp.tile([128, HC, GA], bf, tag="h1")
                sb0 = gp.tile([128, HC, GA], bf, tag="sb0")
                sb1 = gp.tile([128, HC, GA], bf, tag="sb1")
                dz0 = gp.tile([128, HC, GA], bf, tag="dz0")
                ps_oc = ps1.tile([128, 2], f32, tag="ps_oc")

                for m in range(HC):
                    ps_z0 = ps2.tile([128, GA], f32, tag="ps_z0")
                    nc.tensor.matmul(
                        ps_z0[:], w0[:, (e * HC + m) * 128 : (e * HC + m + 1) * 128],
                        X, start=True, stop=True)
                    nc.scalar.activation(h0[:, m, :], ps_z0[:], AF.Tanh,
                                         bias=b0[:, e * HC + m : e * HC + m + 1])
                for m in range(HC):
                    ps_z1 = ps2.tile([128, GA], f32, tag="ps_z1")
                    for kc in range(HC):
                        col = ((e * HC + kc) * HC + m) * 128
                        nc.tensor.matmul(ps_z1[:], w1[:, col : col + 128],
                                         h0[:, kc, :],
                                         start=(kc == 0), stop=(kc == HC - 1))
                    nc.scalar.activation(h1[:, m, :], ps_z1[:], AF.Tanh,
                                         bias=b1[:, e * HC + m : e * HC + m + 1])
                for tt in range(2):
                    for m in range(HC):
                        nc.tensor.matmul(ps_oc[:, tt : tt + 1],
                                         h1[:, m, tt * 128 : (tt + 1) * 128],
                                         w2c[:, e * HC + m : e * HC + m + 1],
                                         start=(m == 0), stop=(m == HC - 1),
                                         skip_group_check=True)
                nc.vector.tensor_scalar(out=o_col[:, 2 * g_i : 2 * g_i + 2],
                                        in0=ps_oc[:], scalar1=b2[:, e : e + 1],
                                        scalar2=None, op0=OP.add)
                for m in range(HC):
                    nc.vector.tensor_tensor(out=sb1[:, m, :], in0=h1[:, m, :],
                                            in1=h1[:, m, :], op=OP.mult)
                    nc.vector.tensor_scalar(out=sb1[:, m, :], in0=sb1[:, m, :],
                                            scalar1=-1.0, scalar2=1.0,
                                            op0=OP.mult, op1=OP.add)
                    nc.vector.tensor_tensor(out=sb0[:, m, :], in0=h0[:, m, :],
                                            in1=h0[:, m, :], op=OP.mult)
                    nc.vector.tensor_scalar(out=sb0[:, m, :], in0=sb0[:, m, :],
                                            scalar1=-1.0, scalar2=1.0,
                                            op0=OP.mult, op1=OP.add)
                for m in range(HC):
                    ps_dh = ps1.tile([128, GA], f32, tag="ps_dh")
                    for kc in range(HC):
                        col = ((e * HC + kc) * HC + m) * 128
                        nc.tensor.matmul(ps_dh[:], w1wt[:, col : col + 128],
                                         sb1[:, kc, :],
                                         start=(kc == 0), stop=(kc == HC - 1))
                    nc.vector.tensor_tensor(out=dz0[:, m, :], in0=ps_dh[:],
                                            in1=sb0[:, m, :], op=OP.mult)
                ps_g = ps1.tile([128, GA], f32, tag="ps_bw")
                for kc in range(HC):
                    nc.tensor.matmul(
                        ps_g[:], w0t[:, (e * HC + kc) * 128 : (e * HC + kc + 1) * 128],
                        dz0[:, kc, :], start=(kc == 0), stop=(kc == HC - 1))
                hh_ = 0 if a0 < SPLIT_AP else 1
                gb_ = a0 - (0 if hh_ == 0 else SPLIT_AP)
                nc.scalar.activation(g_h[hh_][:, gb_ : gb_ + GA], ps_g[:],
                                     AF.Copy)
                for tt in range(2):
                    t_idx = 2 * g_i + tt
                    onehot = gp.tile([128, IMG], bf, tag="onehot")
                    nc.vector.tensor_scalar(out=onehot[:], in0=iota[:],
                                            scalar1=imgs[:, t_idx : t_idx + 1],
                                            scalar2=None, op0=OP.is_equal)
                    nc.tensor.matmul(ps_E[:], o_col[:, t_idx : t_idx + 1],
                                     onehot[:], start=(t_idx == 0),
                                     stop=(t_idx == T - 1), skip_group_check=True)

            esb = pp.tile([1, IMG], f32, tag="esb")
            nc.vector.tensor_copy(esb[:], ps_E[:])
            nc.sync.dma_start(energy_out.ap(), esb[:])

            # ---- routing ----
            ls0a = pload("ls0a", ls0a_d, [128, PADN], i16, pool=ixp, tag="idx")
            ls0b = pload("ls0b", ls0b_d, [128, PADN], i16, pool=ixp, tag="idx")
            ls1i = pload("ls1i", ls1_d, [128, 4 * LCTOT], i16, pool=ixp, tag="idx")
            ls2i = pload("ls2i", ls2_d, [128, 4 * TSC * 128], i16, pool=ixp, tag="idx")
            vsl = pload("vsl", vsl_d, [128, SLOTC], bf, tag="vsl")

            compact = pp.tile([128, LCTOT], bf, tag="compact")
            staging = pp.tile([128, 4 * TSC * 128], bf, tag="bigC")
            stagingT = pp.tile([128, 4 * TSC * 128], bf, tag="bigA")
            slots = pp.tile([128, SLOTC], bf, tag="slots")

            hslice = [(0, SPLIT_AP), (SPLIT_AP, PADN)]
            for h in (0, 1):
                lo_, hi_ = hslice[h]
                for sr, lsx in ((0, ls0a), (1, ls0b)):
                    roff = REG_OFF[h * 2 + sr]
                    rlen = LA2h[h] if sr == 0 else LBh[h]
                    nc.gpsimd.local_scatter(
                        out_ap=compact[:, roff : roff + rlen],
                        data_ap=g_h[h][:],
                        idxs_ap=lsx[:, lo_:hi_],
                        channels=128, num_elems=int(rlen),
                        num_idxs=hi_ - lo_)
            for ch in range(4):
                nc.gpsimd.local_scatter(
                    out_ap=staging[:, ch * TSC * 128 : (ch + 1) * TSC * 128],
                    data_ap=compact[:],
                    idxs_ap=ls1i[:, ch * LCTOT : (ch + 1) * LCTOT],
                    channels=128, num_elems=TSC * 128, num_idxs=LCTOT)
            n_st = 4 * TSC
            for b4 in range((n_st + 3) // 4):
                ps_t = ps1.tile([128, 512], bf, tag="ps_bw")
                nts = min(4, n_st - b4 * 4)
                for j in range(nts):
                    st_i = b4 * 4 + j
                    nc.tensor.transpose(
                        ps_t[:, j * 128 : (j + 1) * 128],
                        staging[:, st_i * 128 : (st_i + 1) * 128], ident[:])
                nc.scalar.activation(
                    stagingT[:, b4 * 512 : b4 * 512 + nts * 128],
                    ps_t[:, : nts * 128], AF.Copy)
            for ch in range(4):
                nc.gpsimd.local_scatter(
                    out_ap=slots[:, ch * (SLOTC // 4) : (ch + 1) * (SLOTC // 4)],
                    data_ap=stagingT[:, ch * TSC * 128 : (ch + 1) * TSC * 128],
                    idxs_ap=ls2i[:, ch * TSC * 128 : (ch + 1) * TSC * 128],
                    channels=128, num_elems=SLOTC // 4, num_idxs=TSC * 128)

            import os as _os
            if _os.environ.get("BPNN_DEBUG"):
                for nm, tt_ in (("dbg_g", g_sb), ("dbg_compact", compact),
                                ("dbg_staging", staging),
                                ("dbg_stagingT", stagingT), ("dbg_slots", slots)):
                    dd = nc.dram_tensor(nm, list(tt_[:].shape), tt_[:].dtype,
                                        kind="ExternalOutput")
                    nc.sync.dma_start(dd.ap(), tt_[:])
            partial = pp.tile([128, QC], f32, tag="partial")
            SQ = SLOTC // 4
            QQ = QC // 4
            for ch in range(4):
                sl_ = slots[:, ch * SQ : (ch + 1) * SQ]
                nc.vector.tensor_tensor(out=sl_, in0=sl_,
                                        in1=vsl[:, ch * SQ : (ch + 1) * SQ],
                                        op=OP.mult)
                nc.vector.tensor_reduce(
                    out=partial[:, ch * QQ : (ch + 1) * QQ],
                    in_=sl_.rearrange("p (q r) -> p q r", r=PAD),
                    axis=mybir.AxisListType.X, op=OP.add, negate=True)

            # ---- mini (residual) routing ----
            TS_LIST = cfg["TS_LIST"]
            SMW = cfg["SMW"]
            stg_off = [0]
            for tsj_ in TS_LIST:
                stg_off.append(stg_off[-1] + tsj_ * 128)
            ls1m = pload("ls1m", ls1m_d, [128, P_MINI * LCTOT], i16, pool=ixp, tag="idx")
            ls2m = pload("ls2m", ls2m_d, [128, PAD2 * SMW], i16, pool=ixp, tag="idx")
            v2 = pload("v2", v2_d, [128, PAD2 * QC], bf)
            staging_m = pp.tile([128, SMW], bf, tag="staging_m")
            stagingT_m = pp.tile([128, SMW], bf, tag="stagingT_m")
            slots_m = pp.tile([128, PAD2 * QC], bf, tag="slots_m")
            for j in range(P_MINI):
                nc.gpsimd.local_scatter(
                    out_ap=staging_m[:, stg_off[j] : stg_off[j + 1]],
                    data_ap=compact[:],
                    idxs_ap=ls1m[:, j * LCTOT : (j + 1) * LCTOT],
                    channels=128, num_elems=stg_off[j + 1] - stg_off[j],
                    num_idxs=LCTOT)
            n_mt = SMW // 128
            for b4 in range((n_mt + 3) // 4):
                ps_t = ps1.tile([128, 512], bf, tag="ps_bw")
                nts = min(4, n_mt - b4 * 4)
                for j in range(nts):
                    st_i = b4 * 4 + j
                    nc.tensor.transpose(
                        ps_t[:, j * 128 : (j + 1) * 128],
                        staging_m[:, st_i * 128 : (st_i + 1) * 128], ident[:])
                nc.scalar.activation(
                    stagingT_m[:, b4 * 512 : b4 * 512 + nts * 128],
                    ps_t[:, : nts * 128], AF.Copy)
            for r in range(PAD2):
                nc.gpsimd.local_scatter(
                    out_ap=slots_m[:, r * QC : (r + 1) * QC],
                    data_ap=stagingT_m[:],
                    idxs_ap=ls2m[:, r * SMW : (r + 1) * SMW],
                    channels=128, num_elems=QC, num_idxs=SMW)
            nc.vector.tensor_tensor(out=slots_m[:], in0=slots_m[:], in1=v2[:],
                                    op=OP.mult)
            partial2 = pp.tile([128, QC], f32, tag="partial2")
            nc.vector.tensor_reduce(
                out=partial2[:],
                in_=slots_m[:].rearrange("p (r q) -> p q r", r=PAD2),
                axis=mybir.AxisListType.X, op=OP.add, negate=True)
            nc.vector.tensor_tensor(out=partial[:], in0=partial[:],
                                    in1=partial2[:], op=OP.add)

            if _os.environ.get("BPNN_DEBUG"):
                for nm2, tt2 in (("dbg_partial", partial),
                                 ("dbg_partial2", partial2),
                                 ("dbg_slots_m", slots_m)):
                    dd = nc.dram_tensor(nm2, list(tt2[:].shape), tt2[:].dtype,
                                        kind="ExternalOutput")
                    nc.sync.dma_start(dd.ap(), tt2[:])

            i_pw = nc.sync.dma_start(
                part_dram.ap().rearrange("(p q) o -> p (q o)", p=128),
                partial[:])
            prev = i_pw

            if no_collective:
                i_out = nc.sync.dma_start(
                    forces_out.ap(),
                    part_dram.ap().rearrange("(p q) o -> p (q o)", p=128)[
                        : 128 // num_devices, :])
                add_dep_helper(i_out.ins, prev.ins, True, "out after partial")
            else:
                i_cc = nc.gpsimd.collective_compute(
                    "ReduceScatter", OP.add,
                    replica_groups=[list(range(num_devices))],
                    ins=[part_dram.ap()], outs=[ar_out.ap()])
                add_dep_helper(i_cc.ins, prev.ins, True, "rs after partial")
                i_out = nc.sync.dma_start(
                    forces_out.ap(),
                    ar_out.ap().rearrange("(p q) o -> p (q o)",
                                          p=128 // num_devices))
                add_dep_helper(i_out.ins, i_cc.ins, True, "out after rs")

    nc.compile()
    return nc


def _assemble(cfg, results):
    N, NI, QC = cfg["N"], cfg["NI"], cfg["QC"]
    energy = np.zeros(NI, dtype=np.float32)
    for k, cm in enumerate(cfg["core_meta"]):
        e = results[k]["energy_out"].reshape(-1)
        energy[cm["img_base"] : cm["img_base"] + cm["n_img"]] = e[: cm["n_img"]]
    ar = np.concatenate([results[k]["forces_out"] for k in range(NC)], axis=0)
    cidx = np.arange(3 * N)
    forces = ar[cidx % 128, cidx // 128].reshape(N, 3).astype(np.float32)
    return energy, forces


_LAST_EXEC_NS = None


def kernel(**inputs):
    global _LAST_EXEC_NS
    import os
    cfg = _host_prepare(inputs)
    nc = _build(cfg)
    trace = bool(os.environ.get("BPNN_TRACE"))
    res = run_bass_kernel_spmd(nc, cfg["in_maps"], list(range(NC)), trace=trace)
    _LAST_EXEC_NS = res.exec_time_ns
    return _assemble(cfg, res.results)
